# revision 14
# baseline (speedup 1.0000x reference)
"""Trainium2 Bass kernel for nn_Attention_41755672052568.

Self-attention block on x:(16,512,32,32):
  GroupNorm(32,eps=1e-6,affine) -> q,k,v = 1x1 convs -> softmax(q^T k / sqrt(C))
  -> out = attn @ v -> 1x1 conv proj -> + residual

Strategy: data-parallel over batch B=16 across 8 NeuronCores (2 samples/core).
Per sample everything is kept on-chip.  v2: all GEMMs run in fp8-e4m3 with
perf_mode=DoubleRow (2 contraction rows per cycle, ~2.2x the bf16 matmul
stream rate measured on HW).  The output norm is dominated by the residual
(attention path is ~6.7% of ||out||), so the 2e-2 rel-err budget allows ~30%
error on the attention path; fp8 costs ~5%.

  - GroupNorm stats via bn_stats/bn_aggr + tiny mask-matmuls for the
    cross-partition group reduce/expand; normalize+cast to fp8 on DVE.
  - Power-of-2 scales are folded into the fp8 weights so every fp8 tensor is
    centered in e4m3's normal range: wq*=256 (incl. the softmax 1/sqrt(C)),
    wk/wv/wp*=16.  The inverse scales are folded into the exp ACT scale
    (2^-12) and the final residual-add multiplier (2^-8).
  - softmax: exp(E*2^-12 - 2) on ACT (the -2 shift guards the fp8 max of
    240 and cancels in the rowsum normalization); row sums via a ones-matmul,
    reciprocal_approx_fast on DVE, applied at the O-GEMM evacuation.
  - The two samples' phases are interleaved (qkv(0), E(0), qkv(1), O(0),
    E(1), O(1)) so sample 1's matmuls cover sample 0's softmax ACT chain.
  - gn affine and biases folded into weights/biases on the host.
"""

import numpy as np
import ml_dtypes

B, C, HW = 16, 512, 1024
NCORES = 8
SPC = B // NCORES  # samples per core
P = 128
CT = C // P        # channel tiles (4)
JT = HW // P       # j tiles (8)
NH = HW // 512     # free-dim halves (2)
GS = 16            # channels per group (512/32)
GPT = P // GS      # groups per channel-tile (8)
EPS = 1e-6

SQ2 = 256.0        # fp8 scale on W2 = wk_eff^T wq_eff (incl. C**-0.5)
SV = 16.0          # fp8 scale on wv
SP = 16.0          # fp8 scale on wp
ESCALE = 1.0 / SQ2         # exp ACT scale
OSCALE = 1.0 / (SP * SV)   # final evacuation multiplier
CSHIFT = 2.0       # exp shift: exp(E - CSHIFT); cancels in normalization

_CACHE = {}

def _make_bacc(bacc, mybir):
    """Bacc subclass that pins Ln and Exp to the combined
    natural_log_exp_and_others ACT table set, so the whole kernel needs a
    single ACT_TABLE_LOAD instead of thrashing between the ln and exp sets.
    Only set *membership* used for placement is edited; set ids keep their
    act_info.json indices, and the combined set physically contains both
    functions, so lowering stays correct."""
    class PinnedActBacc(bacc.Bacc):
        def insert_act_table_loads(self):
            from concourse.hw_specs import get_activation_tables
            import concourse.bacc as _bm
            has_activation = any(
                isinstance(i, mybir.InstActivation)
                for b in self.main_func.blocks
                for i in b.instructions)
            if not has_activation:
                return
            AF = mybir.ActivationFunctionType
            tables = list(get_activation_tables(self.m.arch).items())
            edited = []
            for n, fns in tables:
                if n != "natural_log_exp_and_others":
                    fns = {f for f in fns if f not in (AF.Ln, AF.Exp)}
                edited.append((n, set(fns)))
            _bm._bass_rust.insert_act_table_loads(self, edited)
    return PinnedActBacc


def _emit_consts(nc, tc, const, dram, mybir):
    f32 = mybir.dt.float32
    f8 = mybir.dt.float8e4
    t = {}
    t["wu_sb"] = const.tile([P, CT, C], f8, name="wu_sb")
    t["wv_sb"] = const.tile([P, CT, C], f8, name="wv_sb")
    t["wp_sb"] = const.tile([P, CT, C], f8, name="wp_sb")
    for sb, name in ((t["wu_sb"], "wuT"), (t["wv_sb"], "wvT"),
                     (t["wp_sb"], "wpT")):
        # deprioritized: the first sample's x DMA + stats are the critical
        # path at startup; weights are not needed until the first matmul.
        with tc.high_priority(offset=-500000):
            nc.sync.dma_start(
                sb[:], dram[name].ap().rearrange("(t p) c -> p t c", p=P))
    t["gmask_sb"] = const.tile([P, GPT], f32, name="gmask_sb")
    nc.sync.dma_start(t["gmask_sb"][:], dram["gmask"].ap())
    t["gexp_sb"] = const.tile([P, P], f32, name="gexp_sb")
    nc.sync.dma_start(t["gexp_sb"][:], dram["gexpand"].ap())
    t["ones_sb"] = const.tile([P, 2, P], f8, name="ones_sb")
    nc.vector.memset(t["ones_sb"][:], 1.0)
    t["eps_sb"] = const.tile([P, 1], f32, name="eps_sb")
    nc.vector.memset(t["eps_sb"][:], EPS)
    t["zero_sb"] = const.tile([P, 1], f32, name="zero_sb")
    nc.vector.memset(t["zero_sb"][:], 0.0)
    t["cshift_sb"] = const.tile([P, 1], f32, name="cshift_sb")
    nc.vector.memset(t["cshift_sb"][:], -CSHIFT)
    t["warm_sb"] = const.tile([P, 1], f32, name="warm_sb")
    nc.scalar.activation(t["warm_sb"][:], t["eps_sb"][:],
                         mybir.ActivationFunctionType.Ln,
                         bias=t["eps_sb"][:], scale=1.0)
    t["smus0"] = const.tile([P, 2 * CT], f32, name="smus0")
    nc.vector.memset(t["smus0"][:], 0.0)
    t["smus1"] = const.tile([P, 2 * CT], f32, name="smus1")
    nc.vector.memset(t["smus1"][:], 0.0)
    return t


def _emit_body(nc, tc, pools, cst, dram, mybir, with_bp):
    """One full pass over this core's SPC samples."""
    f32 = mybir.dt.float32
    f8 = mybir.dt.float8e4
    AF = mybir.ActivationFunctionType
    OP = mybir.AluOpType
    DR = mybir.MatmulPerfMode.DoubleRow

    (xpool, xnpool, qkpool, vtpool, atpool, rpool, onpool, outpool, stats,
     psum) = pools

    x_in = dram["x"]
    out_dram = dram["out"]

    x_sbs = []
    xn_sbs = []
    q_sbs, k_sbs, vt_sbs, at_sbs = {}, {}, {}, {}

    # ---------- phase A: load + groupnorm stats + normalize to fp8 ----------
    # Engine split: sample 0's per-channel stats run on ACT (idle during the
    # previous iteration's tail, so the loop boundary chain is short);
    # sample 1's run on DVE bn_stats.  xn normalize is split DVE/GPSIMD.
    def emit_phase_a(s):
        x_sb = xpool.tile([P, CT, HW], f32, tag=f"x{s}")
        x_src = x_in.ap()[s].rearrange("(t p) j -> p t j", p=P)
        for xc in range(2):
            nc.sync.dma_start(x_sb[:, 2 * xc:2 * xc + 2, :],
                              x_src[:, 2 * xc:2 * xc + 2, :])
        x_sbs.append(x_sb)

        # per-channel (mean, E[x^2]) into stats_all[:, t, :]
        stats_all = stats.tile([P, CT, 2], f32, tag=f"stats_all{s}")
        if s == 0:
            for t in range(CT):
                ascr = stats.tile([P, HW], f32, tag="ascr")
                nc.scalar.activation(
                    ascr[:], x_sb[:, t, :], AF.Identity,
                    bias=cst["zero_sb"][:], scale=1.0 / HW,
                    accum_out=stats_all[:, t, 0:1])
                nc.scalar.activation(
                    ascr[:], x_sb[:, t, :], AF.Square,
                    bias=cst["zero_sb"][:], scale=1.0 / 32.0,
                    accum_out=stats_all[:, t, 1:2])
        else:
            for t in range(CT):
                bnst = stats.tile([P, 2, 6], f32, tag="bnst")
                xin = x_sb[:, t, :].rearrange("p (c f) -> p c f", f=512)
                for h in range(2):
                    nc.vector.bn_stats(out=bnst[:, h, :], in_=xin[:, h, :])
                nc.vector.bn_aggr(out=stats_all[:, t, :], in_=bnst[:])
            # bn_aggr gives (mean, var); fix var -> E[x^2]
            m2 = stats.tile([P, CT], f32, tag="m2")
            nc.vector.tensor_tensor(
                m2[:], stats_all[:, :, 0], stats_all[:, :, 0], OP.mult)
            nc.vector.tensor_tensor(
                stats_all[:, :, 1], stats_all[:, :, 1], m2[:], OP.add)
        # group-average across partitions: [8, CT, 2] = (mean_g, Ex2_g)
        gps = psum.tile([GPT, CT, 2], f32, tag="ps")
        nc.tensor.matmul(gps[:], cst["gmask_sb"][:], stats_all[:],
                         start=True, stop=True)
        gm2 = stats.tile([GPT, CT], f32, tag="gm2")
        nc.scalar.activation(gm2[:], gps[:, :, 0], AF.Square,
                             bias=cst["zero_sb"][0:GPT, :], scale=1.0)
        varg = stats.tile([GPT, CT], f32, tag="varg")
        nc.vector.tensor_tensor(varg[:], gps[:, :, 1], gm2[:], OP.subtract)
        # s_g = rsqrt(var+eps) = exp(-0.5*ln(var+eps));  mus_g = mean_g*s_g
        lnv = stats.tile([GPT, CT], f32, tag="lnv")
        nc.scalar.activation(lnv[:], varg[:], AF.Ln,
                             bias=cst["eps_sb"][0:GPT, :], scale=1.0)
        smus = cst[f"smus{s}"]
        nc.scalar.activation(smus[0:GPT, 0:CT], lnv[:], AF.Exp,
                             bias=cst["zero_sb"][0:GPT, :], scale=-0.5)
        nc.vector.scalar_tensor_tensor(
            smus[0:GPT, CT:2 * CT], gps[:, :, 0], 1.0,
            smus[0:GPT, 0:CT], OP.mult, OP.mult)
        # expand group -> channel: chan[p, t]=s, chan[p, CT+t]=mu*s
        cps = psum.tile([P, 2 * CT], f32, tag="ps")
        nc.tensor.matmul(cps[:], cst["gexp_sb"][:], smus[:],
                         start=True, stop=True)
        chan = stats.tile([P, 2 * CT], f32, tag=f"chan{s}")
        if s == 0:
            nc.scalar.activation(chan[:], cps[:], AF.Identity,
                                 bias=cst["zero_sb"][:], scale=1.0)
        else:
            nc.vector.tensor_copy(chan[:], cps[:])

        # normalize + cast to fp8: xn = x*s - mu*s (DVE tiles 0-1 at 2x
        # SBUF rate, GPSIMD tiles 2-3 -- three-way engine spread)
        xn_sb = xnpool.tile([P, CT, HW], f8, tag=f"xn{s}")
        for t in range(CT):
            eng = nc.vector if t < 2 else nc.gpsimd
            eng.tensor_scalar(
                out=xn_sb[:, t, :], in0=x_sb[:, t, :],
                scalar1=chan[:, t:t + 1], scalar2=chan[:, CT + t:CT + t + 1],
                op0=OP.mult, op1=OP.subtract)
        xn_sbs.append(xn_sb)

    # ---------- phase B stages (all GEMMs fp8 DoubleRow) ----------
    def emit_qkv(s):
        # u = W2^T xn with W2 = wk_eff^T wq_eff folded on the host, so
        # E = xn^T W2 xn = u^T xn needs neither q nor k (the k bias cancels
        # in softmax; zero q bias asserted on the host).  u evac on ACT.
        xn_sb = xn_sbs[s]
        u_sb = qkpool.tile([P, CT, HW], f8, tag=f"u{s}")
        q_sbs[s] = u_sb
        for m in range(CT):
            ps = psum.tile([P, HW], f32, tag="ps")
            for n in range(NH):
                for kp in range(CT // 2):
                    nc.tensor.matmul(
                        ps[:, n * 512:(n + 1) * 512],
                        cst["wu_sb"][:, 2 * kp:2 * kp + 2, m * P:(m + 1) * P],
                        xn_sb[:, 2 * kp:2 * kp + 2,
                              n * 512:(n + 1) * 512],
                        start=(kp == 0), stop=(kp == CT // 2 - 1),
                        perf_mode=DR)
            nc.scalar.activation(u_sb[:, m, :], ps[:], AF.Identity,
                                 bias=cst["zero_sb"][:], scale=1.0)

        # vT[j, c] = xn^T @ wv^T (v bias folded into bp)
        vt_sb = vtpool.tile([P, JT, C], f8, tag=f"vt{s}")
        vt_sbs[s] = vt_sb
        for mjp in range(JT // 2):
            ps = psum.tile([P, HW], f32, tag="ps")
            psv = ps[:].rearrange("p (h c) -> p h c", h=2)
            for h in range(2):
                mj = 2 * mjp + h
                for kp in range(CT // 2):
                    nc.tensor.matmul(
                        psv[:, h, :],
                        xn_sb[:, 2 * kp:2 * kp + 2, mj * P:(mj + 1) * P],
                        cst["wv_sb"][:, 2 * kp:2 * kp + 2, :],
                        start=(kp == 0), stop=(kp == CT // 2 - 1),
                        perf_mode=DR)
            nc.vector.tensor_copy(vt_sb[:, 2 * mjp:2 * mjp + 2, :], psv[:])

    def emit_e(s):
        # S[j,i] = E^T = u^T xn; at = exp(E*ESCALE - CSHIFT) in fp8
        u_sb, xn_sb = q_sbs[s], xn_sbs[s]
        at_sb = atpool.tile([P, JT, HW], f8, tag=f"at{s}")
        at_sbs[s] = at_sb
        for mj in range(JT):
            ps = psum.tile([P, HW], f32, tag="ps")
            for n in range(NH):
                for kp in range(CT // 2):
                    nc.tensor.matmul(
                        ps[:, n * 512:(n + 1) * 512],
                        u_sb[:, 2 * kp:2 * kp + 2, mj * P:(mj + 1) * P],
                        xn_sb[:, 2 * kp:2 * kp + 2, n * 512:(n + 1) * 512],
                        start=(kp == 0), stop=(kp == CT // 2 - 1),
                        perf_mode=DR)
            nc.scalar.activation(at_sb[:, mj, :], ps[:], AF.Exp,
                                 bias=cst["cshift_sb"][:], scale=ESCALE)

    def emit_rop(s):
        at_sb, vt_sb, x_sb = at_sbs[s], vt_sbs[s], x_sbs[s]
        # row sums r_i replicated over partitions; rinv = 1/r on DVE
        rinv_sb = rpool.tile([P, HW], f32, tag="rinv")
        ps = psum.tile([P, HW], f32, tag="ps")
        for n in range(NH):
            for kjp in range(JT // 2):
                nc.tensor.matmul(
                    ps[:, n * 512:(n + 1) * 512], cst["ones_sb"][:],
                    at_sb[:, 2 * kjp:2 * kjp + 2, n * 512:(n + 1) * 512],
                    start=(kjp == 0), stop=(kjp == JT // 2 - 1),
                    perf_mode=DR)
        nc.vector.reciprocal_approx_fast(out=rinv_sb[:], in_=ps[:])

        # O GEMM + normalize (on = SV * O_norm in fp8)
        on_sb = onpool.tile([P, CT, HW], f8, tag=f"on{s}")
        for mc in range(CT):
            ps = psum.tile([P, HW], f32, tag="ps")
            for n in range(NH):
                for kjp in range(JT // 2):
                    nc.tensor.matmul(
                        ps[:, n * 512:(n + 1) * 512],
                        vt_sb[:, 2 * kjp:2 * kjp + 2, mc * P:(mc + 1) * P],
                        at_sb[:, 2 * kjp:2 * kjp + 2, n * 512:(n + 1) * 512],
                        start=(kjp == 0), stop=(kjp == JT // 2 - 1),
                        perf_mode=DR)
            nc.vector.tensor_tensor(
                on_sb[:, mc, :], ps[:], rinv_sb[:], OP.mult)

        # proj GEMM + residual (+ bias only when nonzero)
        out_sb = outpool.tile([P, CT, HW], f32, tag=f"out{s}")
        for m in range(CT):
            ps = psum.tile([P, HW], f32, tag="ps")
            for n in range(NH):
                for kp in range(CT // 2):
                    nc.tensor.matmul(
                        ps[:, n * 512:(n + 1) * 512],
                        cst["wp_sb"][:, 2 * kp:2 * kp + 2, m * P:(m + 1) * P],
                        on_sb[:, 2 * kp:2 * kp + 2, n * 512:(n + 1) * 512],
                        start=(kp == 0), stop=(kp == CT // 2 - 1),
                        perf_mode=DR)
            if with_bp:
                tmp = rpool.tile([P, HW], f32, tag="bptmp")
                nc.vector.tensor_scalar(
                    out=tmp[:], in0=ps[:], scalar1=OSCALE,
                    scalar2=cst["bp_sb"][:, m:m + 1],
                    op0=OP.mult, op1=OP.add)
                nc.vector.tensor_tensor(
                    out_sb[:, m, :], tmp[:], x_sb[:, m, :], OP.add)
            else:
                nc.vector.scalar_tensor_tensor(
                    out_sb[:, m, :], ps[:], OSCALE, x_sb[:, m, :],
                    OP.mult, OP.add)
        out_dst = out_dram.ap()[s].rearrange("(t p) j -> p t j", p=P)
        for mo in range(0, CT, 2):
            nc.sync.dma_start(out_dst[:, mo:mo + 2, :],
                              out_sb[:, mo:mo + 2, :])

    # interleaved schedule: sample 1's matmuls cover sample 0's softmax ACT.
    # phase A(1) is emitted after e(0) so its tiny group-stats matmuls don't
    # sit in PE program order ahead of sample 0's GEMMs.
    emit_phase_a(0)
    emit_qkv(0)
    emit_e(0)
    emit_phase_a(1)
    emit_qkv(1)
    emit_rop(0)
    emit_e(1)
    emit_rop(1)


def _build_nc(loop_reps=None, with_bp=False):
    import concourse.bacc as bacc
    import concourse.tile as tile
    import concourse.mybir as mybir

    f32 = mybir.dt.float32
    f8 = mybir.dt.float8e4

    nc = _make_bacc(bacc, mybir)("TRN2", target_bir_lowering=False,
                                  debug=False, num_devices=NCORES)

    dram = {
        "x": nc.dram_tensor("x", [SPC, C, HW], f32, kind="ExternalInput"),
        "wuT": nc.dram_tensor("wuT", [C, C], f8, kind="ExternalInput"),
        "wvT": nc.dram_tensor("wvT", [C, C], f8, kind="ExternalInput"),
        "wpT": nc.dram_tensor("wpT", [C, C], f8, kind="ExternalInput"),
        "gmask": nc.dram_tensor("gmask", [P, GPT], f32, kind="ExternalInput"),
        "gexpand": nc.dram_tensor("gexpand", [P, P], f32,
                                  kind="ExternalInput"),
        "out": nc.dram_tensor("out", [SPC, C, HW], f32,
                              kind="ExternalOutput"),
    }
    if with_bp:
        dram["bp"] = nc.dram_tensor("bp", [P, CT], f32, kind="ExternalInput")

    from contextlib import ExitStack

    with tile.TileContext(nc) as tc:
        with ExitStack() as ctx:
            const = ctx.enter_context(tc.tile_pool(name="const", bufs=1))
            pools = (
                ctx.enter_context(tc.tile_pool(name="xp", bufs=1)),
                ctx.enter_context(tc.tile_pool(name="xnp", bufs=1)),
                ctx.enter_context(tc.tile_pool(name="qkp", bufs=1)),
                ctx.enter_context(tc.tile_pool(name="vtp", bufs=1)),
                ctx.enter_context(tc.tile_pool(name="atp", bufs=1)),
                ctx.enter_context(tc.tile_pool(name="rp", bufs=2)),
                ctx.enter_context(tc.tile_pool(name="onp", bufs=1)),
                ctx.enter_context(tc.tile_pool(name="outp", bufs=2)),
                ctx.enter_context(tc.tile_pool(name="stats", bufs=2)),
                ctx.enter_context(tc.tile_pool(name="psum", bufs=4,
                                               space="PSUM")),
            )
            cst = _emit_consts(nc, tc, const, dram, mybir)
            if with_bp:
                cst["bp_sb"] = const.tile([P, CT], f32, name="bp_sb")
                nc.sync.dma_start(cst["bp_sb"][:], dram["bp"].ap())
            if loop_reps is None:
                _emit_body(nc, tc, pools, cst, dram, mybir, with_bp)
            else:
                with tc.For_i(0, loop_reps, 1):
                    _emit_body(nc, tc, pools, cst, dram, mybir, with_bp)

    nc.compile()
    return nc


def get_nc(loop_reps=None, with_bp=False):
    key = ("nc", loop_reps, with_bp)
    if key not in _CACHE:
        _CACHE[key] = _build_nc(loop_reps, with_bp)
    return _CACHE[key]


def _to_f8(a):
    return np.ascontiguousarray(
        np.clip(a, -240.0, 240.0)).astype(ml_dtypes.float8_e4m3)


def make_in_maps(x, gn_gamma, gn_beta, wq, bq, wk, bk, wv, bv, wp, bp):
    x = np.asarray(x, np.float32).reshape(B, C, HW)
    gamma = np.asarray(gn_gamma, np.float64)
    beta = np.asarray(gn_beta, np.float64)
    wq = np.asarray(wq, np.float64)
    wk = np.asarray(wk, np.float64)
    wv = np.asarray(wv, np.float64)
    wp = np.asarray(wp, np.float64)
    bq = np.asarray(bq, np.float64)
    bk = np.asarray(bk, np.float64)
    bv = np.asarray(bv, np.float64)
    bp = np.asarray(bp, np.float64)

    scale = C ** -0.5
    wq_eff = (wq * gamma[None, :]) * scale
    bq_eff = (wq @ beta + bq) * scale
    wk_eff = wk * gamma[None, :]
    wv_eff = wv * gamma[None, :] * SV
    bv_eff = wv @ beta + bv
    wp_eff = wp * SP
    bp_eff = wp @ bv_eff + bp

    # E = q^T k: the k bias cancels in softmax; with zero q bias the whole
    # logit matrix is xn^T W2 xn with W2 = wk_eff^T wq_eff folded on host,
    # computed on-chip as u = W2^T xn (weights slot holds W2 directly, not
    # transposed), then S = u^T xn.
    if np.abs(bq_eff).max() > 1e-12:
        raise NotImplementedError("nonzero q bias not supported by the "
                                  "folded-W2 attention path")
    W2 = (wk_eff.T @ wq_eff) * SQ2
    wuT = _to_f8(W2)
    wvT = _to_f8(wv_eff.T)
    wpT = _to_f8(wp_eff.T)
    with_bp = bool(np.any(np.abs(bp_eff) > 0))
    bpp = np.ascontiguousarray(bp_eff.reshape(CT, P).T).astype(np.float32)

    gmask = np.zeros((P, GPT), np.float32)
    for p_ in range(P):
        gmask[p_, p_ // GS] = 1.0 / GS
    gexpand = np.zeros((P, P), np.float32)
    for p_ in range(P):
        gexpand[p_ // GS, p_] = 1.0

    in_maps = []
    for c in range(NCORES):
        m = {
            "x": np.ascontiguousarray(x[c * SPC:(c + 1) * SPC]),
            "wuT": wuT, "wvT": wvT, "wpT": wpT,
            "gmask": gmask, "gexpand": gexpand,
        }
        if with_bp:
            m["bp"] = bpp
        in_maps.append(m)
    return in_maps, with_bp


def kernel(**inputs):
    from concourse.bass_utils import run_bass_kernel_spmd

    in_maps, with_bp = make_in_maps(**inputs)
    nc = get_nc(with_bp=with_bp)
    res = run_bass_kernel_spmd(nc, in_maps, core_ids=list(range(NCORES)))
    out = np.concatenate([r["out"] for r in res.results], axis=0)
    return np.ascontiguousarray(out.reshape(B, C, 32, 32), dtype=np.float32)


# Pre-build the bass program at import (host-side only, no device access) so
# the first kernel() call doesn't pay the ~1 s IR build.  Safe to fail: the
# build is retried lazily inside kernel() via get_nc().
try:
    get_nc()
except Exception:  # noqa: BLE001
    _CACHE.pop(("nc", None, False), None)


# revision 16
# speedup vs baseline: 2.2870x; 2.2870x over previous
"""Trainium2 Bass kernel for nn_Attention_41755672052568.

Self-attention block on x:(16,512,32,32):
  GroupNorm(32,eps=1e-6,affine) -> q,k,v = 1x1 convs -> softmax(q^T k / sqrt(C))
  -> out = attn @ v -> 1x1 conv proj -> + residual

Strategy: data-parallel over batch B=16 across 8 NeuronCores (2 samples/core).
Per sample everything is kept on-chip.  v2: all GEMMs run in fp8-e4m3 with
perf_mode=DoubleRow (2 contraction rows per cycle, ~2.2x the bf16 matmul
stream rate measured on HW).  The output norm is dominated by the residual
(attention path is ~6.7% of ||out||), so the 2e-2 rel-err budget allows ~30%
error on the attention path; fp8 costs ~5%.

  - GroupNorm stats via bn_stats/bn_aggr + tiny mask-matmuls for the
    cross-partition group reduce/expand; normalize+cast to fp8 on DVE.
  - Power-of-2 scales are folded into the fp8 weights so every fp8 tensor is
    centered in e4m3's normal range: wq*=256 (incl. the softmax 1/sqrt(C)),
    wk/wv/wp*=16.  The inverse scales are folded into the exp ACT scale
    (2^-12) and the final residual-add multiplier (2^-8).
  - softmax: exp(E*2^-12 - 2) on ACT (the -2 shift guards the fp8 max of
    240 and cancels in the rowsum normalization); row sums via a ones-matmul,
    reciprocal_approx_fast on DVE, applied at the O-GEMM evacuation.
  - The two samples' phases are interleaved (qkv(0), E(0), qkv(1), O(0),
    E(1), O(1)) so sample 1's matmuls cover sample 0's softmax ACT chain.
  - gn affine and biases folded into weights/biases on the host.
"""

import numpy as np
import ml_dtypes

B, C, HW = 16, 512, 1024
NCORES = 8
SPC = B // NCORES  # samples per core
P = 128
CT = C // P        # channel tiles (4)
JT = HW // P       # j tiles (8)
NH = HW // 512     # free-dim halves (2)
GS = 16            # channels per group (512/32)
GPT = P // GS      # groups per channel-tile (8)
EPS = 1e-6

SQ2 = 256.0        # fp8 scale on W2 = wk_eff^T wq_eff (incl. C**-0.5)
SV = 16.0          # fp8 scale on wv
SP = 16.0          # fp8 scale on wp
ESCALE = 1.0 / SQ2         # exp ACT scale
OSCALE = 1.0 / (SP * SV)   # final evacuation multiplier
CSHIFT = 2.0       # exp shift: exp(E - CSHIFT); cancels in normalization

_CACHE = {}

def _make_bacc(bacc, mybir):
    """Bacc subclass that pins Ln and Exp to the combined
    natural_log_exp_and_others ACT table set, so the whole kernel needs a
    single ACT_TABLE_LOAD instead of thrashing between the ln and exp sets.
    Only set *membership* used for placement is edited; set ids keep their
    act_info.json indices, and the combined set physically contains both
    functions, so lowering stays correct."""
    class PinnedActBacc(bacc.Bacc):
        def insert_act_table_loads(self):
            from concourse.hw_specs import get_activation_tables
            import concourse.bacc as _bm
            has_activation = any(
                isinstance(i, mybir.InstActivation)
                for b in self.main_func.blocks
                for i in b.instructions)
            if not has_activation:
                return
            AF = mybir.ActivationFunctionType
            tables = list(get_activation_tables(self.m.arch).items())
            edited = []
            for n, fns in tables:
                if n != "natural_log_exp_and_others":
                    fns = {f for f in fns if f not in (AF.Ln, AF.Exp)}
                edited.append((n, set(fns)))
            _bm._bass_rust.insert_act_table_loads(self, edited)
    return PinnedActBacc


def _emit_consts(nc, tc, const, dram, mybir):
    f32 = mybir.dt.float32
    f8 = mybir.dt.float8e4
    t = {}
    t["wu_sb"] = const.tile([P, CT, C], f8, name="wu_sb")
    t["wv_sb"] = const.tile([P, CT, C], f8, name="wv_sb")
    t["wp_sb"] = const.tile([P, CT, C], f8, name="wp_sb")
    for sb, name in ((t["wu_sb"], "wuT"), (t["wv_sb"], "wvT"),
                     (t["wp_sb"], "wpT")):
        # deprioritized: the first sample's x DMA + stats are the critical
        # path at startup; weights are not needed until the first matmul.
        with tc.high_priority(offset=-500000):
            nc.sync.dma_start(
                sb[:], dram[name].ap().rearrange("(t p) c -> p t c", p=P))
    t["gmask_sb"] = const.tile([P, GPT], f32, name="gmask_sb")
    nc.sync.dma_start(t["gmask_sb"][:], dram["gmask"].ap())
    t["gexp_sb"] = const.tile([P, P], f32, name="gexp_sb")
    nc.sync.dma_start(t["gexp_sb"][:], dram["gexpand"].ap())
    t["ones_sb"] = const.tile([P, 2, P], f8, name="ones_sb")
    nc.vector.memset(t["ones_sb"][:], 1.0)
    t["eps_sb"] = const.tile([P, 1], f32, name="eps_sb")
    nc.vector.memset(t["eps_sb"][:], EPS)
    t["zero_sb"] = const.tile([P, 1], f32, name="zero_sb")
    nc.vector.memset(t["zero_sb"][:], 0.0)
    t["cshift_sb"] = const.tile([P, 1], f32, name="cshift_sb")
    nc.vector.memset(t["cshift_sb"][:], -CSHIFT)
    t["warm_sb"] = const.tile([P, 1], f32, name="warm_sb")
    nc.scalar.activation(t["warm_sb"][:], t["eps_sb"][:],
                         mybir.ActivationFunctionType.Ln,
                         bias=t["eps_sb"][:], scale=1.0)
    t["smus0"] = const.tile([P, 2 * CT], f32, name="smus0")
    nc.vector.memset(t["smus0"][:], 0.0)
    t["smus1"] = const.tile([P, 2 * CT], f32, name="smus1")
    nc.vector.memset(t["smus1"][:], 0.0)
    return t


def _emit_body(nc, tc, pools, cst, dram, mybir, with_bp):
    """One full pass over this core's SPC samples."""
    f32 = mybir.dt.float32
    f8 = mybir.dt.float8e4
    AF = mybir.ActivationFunctionType
    OP = mybir.AluOpType
    DR = mybir.MatmulPerfMode.DoubleRow

    (xpool, xnpool, qkpool, vtpool, atpool, rpool, onpool, outpool, stats,
     psum) = pools

    x_in = dram["x"]
    out_dram = dram["out"]

    x_sbs = []
    xn_sbs = []
    q_sbs, k_sbs, vt_sbs, at_sbs = {}, {}, {}, {}

    # ---------- phase A: load + groupnorm stats + normalize to fp8 ----------
    # Engine split: sample 0's per-channel stats run on ACT (idle during the
    # previous iteration's tail, so the loop boundary chain is short);
    # sample 1's run on DVE bn_stats.  xn normalize is split DVE/GPSIMD.
    def emit_phase_a(s):
        x_sb = xpool.tile([P, CT, HW], f32, tag=f"x{s}")
        x_src = x_in.ap()[s].rearrange("(t p) j -> p t j", p=P)
        for xc in range(2):
            nc.sync.dma_start(x_sb[:, 2 * xc:2 * xc + 2, :],
                              x_src[:, 2 * xc:2 * xc + 2, :])
        x_sbs.append(x_sb)

        # per-channel (mean, E[x^2]) into stats_all[:, t, :]
        stats_all = stats.tile([P, CT, 2], f32, tag=f"stats_all{s}")
        if s == 0:
            for t in range(CT):
                ascr = stats.tile([P, HW], f32, tag="ascr")
                nc.scalar.activation(
                    ascr[:], x_sb[:, t, :], AF.Identity,
                    bias=cst["zero_sb"][:], scale=1.0 / HW,
                    accum_out=stats_all[:, t, 0:1])
                nc.scalar.activation(
                    ascr[:], x_sb[:, t, :], AF.Square,
                    bias=cst["zero_sb"][:], scale=1.0 / 32.0,
                    accum_out=stats_all[:, t, 1:2])
        else:
            for t in range(CT):
                bnst = stats.tile([P, 2, 6], f32, tag="bnst")
                xin = x_sb[:, t, :].rearrange("p (c f) -> p c f", f=512)
                for h in range(2):
                    nc.vector.bn_stats(out=bnst[:, h, :], in_=xin[:, h, :])
                nc.vector.bn_aggr(out=stats_all[:, t, :], in_=bnst[:])
            # bn_aggr gives (mean, var); fix var -> E[x^2]
            m2 = stats.tile([P, CT], f32, tag="m2")
            nc.vector.tensor_tensor(
                m2[:], stats_all[:, :, 0], stats_all[:, :, 0], OP.mult)
            nc.vector.tensor_tensor(
                stats_all[:, :, 1], stats_all[:, :, 1], m2[:], OP.add)
        # group-average across partitions: [8, CT, 2] = (mean_g, Ex2_g)
        gps = psum.tile([GPT, CT, 2], f32, tag="ps")
        nc.tensor.matmul(gps[:], cst["gmask_sb"][:], stats_all[:],
                         start=True, stop=True)
        gm2 = stats.tile([GPT, CT], f32, tag="gm2")
        nc.scalar.activation(gm2[:], gps[:, :, 0], AF.Square,
                             bias=cst["zero_sb"][0:GPT, :], scale=1.0)
        varg = stats.tile([GPT, CT], f32, tag="varg")
        nc.vector.tensor_tensor(varg[:], gps[:, :, 1], gm2[:], OP.subtract)
        # s_g = rsqrt(var+eps) = exp(-0.5*ln(var+eps));  mus_g = mean_g*s_g
        lnv = stats.tile([GPT, CT], f32, tag="lnv")
        nc.scalar.activation(lnv[:], varg[:], AF.Ln,
                             bias=cst["eps_sb"][0:GPT, :], scale=1.0)
        smus = cst[f"smus{s}"]
        nc.scalar.activation(smus[0:GPT, 0:CT], lnv[:], AF.Exp,
                             bias=cst["zero_sb"][0:GPT, :], scale=-0.5)
        nc.vector.scalar_tensor_tensor(
            smus[0:GPT, CT:2 * CT], gps[:, :, 0], 1.0,
            smus[0:GPT, 0:CT], OP.mult, OP.mult)
        # expand group -> channel: chan[p, t]=s, chan[p, CT+t]=mu*s
        cps = psum.tile([P, 2 * CT], f32, tag="ps")
        nc.tensor.matmul(cps[:], cst["gexp_sb"][:], smus[:],
                         start=True, stop=True)
        chan = stats.tile([P, 2 * CT], f32, tag=f"chan{s}")
        if s == 0:
            nc.scalar.activation(chan[:], cps[:], AF.Identity,
                                 bias=cst["zero_sb"][:], scale=1.0)
        else:
            nc.vector.tensor_copy(chan[:], cps[:])

        # normalize + cast to fp8 on DVE (2x SBUF-read rate; GPSIMD fp8
        # output measured 18us/op -- software path, never use): xn = x*s-mu*s
        xn_sb = xnpool.tile([P, CT, HW], f8, tag=f"xn{s}")
        for t in range(CT):
            nc.vector.tensor_scalar(
                out=xn_sb[:, t, :], in0=x_sb[:, t, :],
                scalar1=chan[:, t:t + 1], scalar2=chan[:, CT + t:CT + t + 1],
                op0=OP.mult, op1=OP.subtract)
        xn_sbs.append(xn_sb)

    # ---------- phase B stages (all GEMMs fp8 DoubleRow) ----------
    def emit_qkv(s):
        # u = W2^T xn with W2 = wk_eff^T wq_eff folded on the host, so
        # E = xn^T W2 xn = u^T xn needs neither q nor k (the k bias cancels
        # in softmax; zero q bias asserted on the host).  u evac on ACT.
        xn_sb = xn_sbs[s]
        u_sb = qkpool.tile([P, CT, HW], f8, tag=f"u{s}")
        q_sbs[s] = u_sb
        for m in range(CT):
            ps = psum.tile([P, HW], f32, tag="ps")
            for n in range(NH):
                for kp in range(CT // 2):
                    nc.tensor.matmul(
                        ps[:, n * 512:(n + 1) * 512],
                        cst["wu_sb"][:, 2 * kp:2 * kp + 2, m * P:(m + 1) * P],
                        xn_sb[:, 2 * kp:2 * kp + 2,
                              n * 512:(n + 1) * 512],
                        start=(kp == 0), stop=(kp == CT // 2 - 1),
                        perf_mode=DR)
            nc.vector.tensor_copy(u_sb[:, m, :], ps[:])

        # vT[j, c] = xn^T @ wv^T (v bias folded into bp)
        vt_sb = vtpool.tile([P, JT, C], f8, tag=f"vt{s}")
        vt_sbs[s] = vt_sb
        for mjp in range(JT // 2):
            ps = psum.tile([P, HW], f32, tag="ps")
            psv = ps[:].rearrange("p (h c) -> p h c", h=2)
            for h in range(2):
                mj = 2 * mjp + h
                for kp in range(CT // 2):
                    nc.tensor.matmul(
                        psv[:, h, :],
                        xn_sb[:, 2 * kp:2 * kp + 2, mj * P:(mj + 1) * P],
                        cst["wv_sb"][:, 2 * kp:2 * kp + 2, :],
                        start=(kp == 0), stop=(kp == CT // 2 - 1),
                        perf_mode=DR)
            nc.vector.tensor_copy(vt_sb[:, 2 * mjp:2 * mjp + 2, :], psv[:])

    def emit_e(s):
        # S[j,i] = E^T = u^T xn; at = exp(E*ESCALE - CSHIFT) in fp8
        u_sb, xn_sb = q_sbs[s], xn_sbs[s]
        at_sb = atpool.tile([P, JT, HW], f8, tag=f"at{s}")
        at_sbs[s] = at_sb
        for mj in range(JT):
            ps = psum.tile([P, HW], f32, tag="ps")
            for n in range(NH):
                for kp in range(CT // 2):
                    nc.tensor.matmul(
                        ps[:, n * 512:(n + 1) * 512],
                        u_sb[:, 2 * kp:2 * kp + 2, mj * P:(mj + 1) * P],
                        xn_sb[:, 2 * kp:2 * kp + 2, n * 512:(n + 1) * 512],
                        start=(kp == 0), stop=(kp == CT // 2 - 1),
                        perf_mode=DR)
            nc.scalar.activation(at_sb[:, mj, :], ps[:], AF.Exp,
                                 bias=cst["cshift_sb"][:], scale=ESCALE)

    def emit_rop(s):
        at_sb, vt_sb, x_sb = at_sbs[s], vt_sbs[s], x_sbs[s]
        # row sums r_i replicated over partitions; rinv = 1/r on DVE
        rinv_sb = rpool.tile([P, HW], f32, tag="rinv")
        ps = psum.tile([P, HW], f32, tag="ps")
        for n in range(NH):
            for kjp in range(JT // 2):
                nc.tensor.matmul(
                    ps[:, n * 512:(n + 1) * 512], cst["ones_sb"][:],
                    at_sb[:, 2 * kjp:2 * kjp + 2, n * 512:(n + 1) * 512],
                    start=(kjp == 0), stop=(kjp == JT // 2 - 1),
                    perf_mode=DR)
        nc.vector.reciprocal_approx_fast(out=rinv_sb[:], in_=ps[:])

        # O GEMM + normalize (on = SV * O_norm in fp8)
        on_sb = onpool.tile([P, CT, HW], f8, tag=f"on{s}")
        for mc in range(CT):
            ps = psum.tile([P, HW], f32, tag="ps")
            for n in range(NH):
                for kjp in range(JT // 2):
                    nc.tensor.matmul(
                        ps[:, n * 512:(n + 1) * 512],
                        vt_sb[:, 2 * kjp:2 * kjp + 2, mc * P:(mc + 1) * P],
                        at_sb[:, 2 * kjp:2 * kjp + 2, n * 512:(n + 1) * 512],
                        start=(kjp == 0), stop=(kjp == JT // 2 - 1),
                        perf_mode=DR)
            nc.vector.tensor_tensor(
                on_sb[:, mc, :], ps[:], rinv_sb[:], OP.mult)

        # proj GEMM + residual (+ bias only when nonzero)
        out_sb = outpool.tile([P, CT, HW], f32, tag=f"out{s}")
        for m in range(CT):
            ps = psum.tile([P, HW], f32, tag="ps")
            for n in range(NH):
                for kp in range(CT // 2):
                    nc.tensor.matmul(
                        ps[:, n * 512:(n + 1) * 512],
                        cst["wp_sb"][:, 2 * kp:2 * kp + 2, m * P:(m + 1) * P],
                        on_sb[:, 2 * kp:2 * kp + 2, n * 512:(n + 1) * 512],
                        start=(kp == 0), stop=(kp == CT // 2 - 1),
                        perf_mode=DR)
            if with_bp:
                tmp = rpool.tile([P, HW], f32, tag="bptmp")
                nc.vector.tensor_scalar(
                    out=tmp[:], in0=ps[:], scalar1=OSCALE,
                    scalar2=cst["bp_sb"][:, m:m + 1],
                    op0=OP.mult, op1=OP.add)
                nc.vector.tensor_tensor(
                    out_sb[:, m, :], tmp[:], x_sb[:, m, :], OP.add)
            else:
                nc.vector.scalar_tensor_tensor(
                    out_sb[:, m, :], ps[:], OSCALE, x_sb[:, m, :],
                    OP.mult, OP.add)
        out_dst = out_dram.ap()[s].rearrange("(t p) j -> p t j", p=P)
        for mo in range(0, CT, 2):
            nc.sync.dma_start(out_dst[:, mo:mo + 2, :],
                              out_sb[:, mo:mo + 2, :])

    # interleaved schedule: sample 1's matmuls cover sample 0's softmax ACT.
    # phase A(1) is emitted after e(0) so its tiny group-stats matmuls don't
    # sit in PE program order ahead of sample 0's GEMMs.
    emit_phase_a(0)
    emit_qkv(0)
    emit_e(0)
    emit_phase_a(1)
    emit_qkv(1)
    emit_rop(0)
    emit_e(1)
    emit_rop(1)


def _build_nc(loop_reps=None, with_bp=False):
    import concourse.bacc as bacc
    import concourse.tile as tile
    import concourse.mybir as mybir

    f32 = mybir.dt.float32
    f8 = mybir.dt.float8e4

    nc = _make_bacc(bacc, mybir)("TRN2", target_bir_lowering=False,
                                  debug=False, num_devices=NCORES)

    dram = {
        "x": nc.dram_tensor("x", [SPC, C, HW], f32, kind="ExternalInput"),
        "wuT": nc.dram_tensor("wuT", [C, C], f8, kind="ExternalInput"),
        "wvT": nc.dram_tensor("wvT", [C, C], f8, kind="ExternalInput"),
        "wpT": nc.dram_tensor("wpT", [C, C], f8, kind="ExternalInput"),
        "gmask": nc.dram_tensor("gmask", [P, GPT], f32, kind="ExternalInput"),
        "gexpand": nc.dram_tensor("gexpand", [P, P], f32,
                                  kind="ExternalInput"),
        "out": nc.dram_tensor("out", [SPC, C, HW], f32,
                              kind="ExternalOutput"),
    }
    if with_bp:
        dram["bp"] = nc.dram_tensor("bp", [P, CT], f32, kind="ExternalInput")

    from contextlib import ExitStack

    with tile.TileContext(nc) as tc:
        with ExitStack() as ctx:
            const = ctx.enter_context(tc.tile_pool(name="const", bufs=1))
            pools = (
                ctx.enter_context(tc.tile_pool(name="xp", bufs=1)),
                ctx.enter_context(tc.tile_pool(name="xnp", bufs=1)),
                ctx.enter_context(tc.tile_pool(name="qkp", bufs=1)),
                ctx.enter_context(tc.tile_pool(name="vtp", bufs=1)),
                ctx.enter_context(tc.tile_pool(name="atp", bufs=1)),
                ctx.enter_context(tc.tile_pool(name="rp", bufs=2)),
                ctx.enter_context(tc.tile_pool(name="onp", bufs=1)),
                ctx.enter_context(tc.tile_pool(name="outp", bufs=2)),
                ctx.enter_context(tc.tile_pool(name="stats", bufs=2)),
                ctx.enter_context(tc.tile_pool(name="psum", bufs=4,
                                               space="PSUM")),
            )
            cst = _emit_consts(nc, tc, const, dram, mybir)
            if with_bp:
                cst["bp_sb"] = const.tile([P, CT], f32, name="bp_sb")
                nc.sync.dma_start(cst["bp_sb"][:], dram["bp"].ap())
            if loop_reps is None:
                _emit_body(nc, tc, pools, cst, dram, mybir, with_bp)
            else:
                with tc.For_i(0, loop_reps, 1):
                    _emit_body(nc, tc, pools, cst, dram, mybir, with_bp)

    nc.compile()
    return nc


def get_nc(loop_reps=None, with_bp=False):
    key = ("nc", loop_reps, with_bp)
    if key not in _CACHE:
        _CACHE[key] = _build_nc(loop_reps, with_bp)
    return _CACHE[key]


def _to_f8(a):
    return np.ascontiguousarray(
        np.clip(a, -240.0, 240.0)).astype(ml_dtypes.float8_e4m3)


def make_in_maps(x, gn_gamma, gn_beta, wq, bq, wk, bk, wv, bv, wp, bp):
    x = np.asarray(x, np.float32).reshape(B, C, HW)
    gamma = np.asarray(gn_gamma, np.float64)
    beta = np.asarray(gn_beta, np.float64)
    wq = np.asarray(wq, np.float64)
    wk = np.asarray(wk, np.float64)
    wv = np.asarray(wv, np.float64)
    wp = np.asarray(wp, np.float64)
    bq = np.asarray(bq, np.float64)
    bk = np.asarray(bk, np.float64)
    bv = np.asarray(bv, np.float64)
    bp = np.asarray(bp, np.float64)

    scale = C ** -0.5
    wq_eff = (wq * gamma[None, :]) * scale
    bq_eff = (wq @ beta + bq) * scale
    wk_eff = wk * gamma[None, :]
    wv_eff = wv * gamma[None, :] * SV
    bv_eff = wv @ beta + bv
    wp_eff = wp * SP
    bp_eff = wp @ bv_eff + bp

    # E = q^T k: the k bias cancels in softmax; with zero q bias the whole
    # logit matrix is xn^T W2 xn with W2 = wk_eff^T wq_eff folded on host,
    # computed on-chip as u = W2^T xn (weights slot holds W2 directly, not
    # transposed), then S = u^T xn.
    if np.abs(bq_eff).max() > 1e-12:
        raise NotImplementedError("nonzero q bias not supported by the "
                                  "folded-W2 attention path")
    W2 = (wk_eff.T @ wq_eff) * SQ2
    wuT = _to_f8(W2)
    wvT = _to_f8(wv_eff.T)
    wpT = _to_f8(wp_eff.T)
    with_bp = bool(np.any(np.abs(bp_eff) > 0))
    bpp = np.ascontiguousarray(bp_eff.reshape(CT, P).T).astype(np.float32)

    gmask = np.zeros((P, GPT), np.float32)
    for p_ in range(P):
        gmask[p_, p_ // GS] = 1.0 / GS
    gexpand = np.zeros((P, P), np.float32)
    for p_ in range(P):
        gexpand[p_ // GS, p_] = 1.0

    in_maps = []
    for c in range(NCORES):
        m = {
            "x": np.ascontiguousarray(x[c * SPC:(c + 1) * SPC]),
            "wuT": wuT, "wvT": wvT, "wpT": wpT,
            "gmask": gmask, "gexpand": gexpand,
        }
        if with_bp:
            m["bp"] = bpp
        in_maps.append(m)
    return in_maps, with_bp


def kernel(**inputs):
    from concourse.bass_utils import run_bass_kernel_spmd

    in_maps, with_bp = make_in_maps(**inputs)
    nc = get_nc(with_bp=with_bp)
    res = run_bass_kernel_spmd(nc, in_maps, core_ids=list(range(NCORES)))
    out = np.concatenate([r["out"] for r in res.results], axis=0)
    return np.ascontiguousarray(out.reshape(B, C, 32, 32), dtype=np.float32)


# Pre-build the bass program at import (host-side only, no device access) so
# the first kernel() call doesn't pay the ~1 s IR build.  Safe to fail: the
# build is retried lazily inside kernel() via get_nc().
try:
    get_nc()
except Exception:  # noqa: BLE001
    _CACHE.pop(("nc", None, False), None)
